# revision 8
# baseline (speedup 1.0000x reference)
"""Distributed Bass kernel for nn_Attention_6287832122083 on 8 TRN2 NeuronCores.

Strategy: tensor-parallel over heads (2 heads per core).
 - Each core computes q,k,v for its 2 heads (f32r matmuls), applies RoPE,
   runs causal attention (skipping fully-masked key blocks), normalizes,
   and AllGathers the per-head attention outputs (bf16).
 - Each core then computes a 256-column slice of the final output through
   wo, producing an output-transposed [256, B*S] slice.
 - Host unshard = concatenate slices + transpose (no arithmetic).
"""

import math
from contextlib import ExitStack

import numpy as np

import concourse.bass as bass
import concourse.bacc as bacc
import concourse.mybir as mybir
import concourse.tile as tile
from concourse import bass_utils

F32 = mybir.dt.float32
F32R = mybir.dt.float32r
BF16 = mybir.dt.bfloat16
NP_BF16 = mybir.dt.np(BF16)
EXP = mybir.ActivationFunctionType.Exp

B, S, D, H = 2, 2048, 2048, 16
HD = D // H              # 128
T = B * S                # 4096 tokens
NCORES = 8
HLOC = H // NCORES       # 2 heads per core
CHK = 256                # QKV token chunk
NCHK_B = S // CHK        # 8 chunks per batch
QC = 512                 # attention query chunk
NQC_B = S // QC          # 4 query chunks per batch
KT = 128                 # key tile
SCALE = 1.0 / math.sqrt(HD)
SQHD = math.sqrt(HD)
NDT = D // 128           # 16 contraction tiles


def build_kernel():
    nc = bacc.Bacc(
        "TRN2",
        target_bir_lowering=False,
        debug=False,
        enable_asserts=False,
        num_devices=NCORES,
    )

    # Per-core DRAM parameters (f32r tensors carry plain fp32 bits).
    xT = nc.dram_tensor("xT", [D, T], F32R, kind="ExternalInput")
    wqp = nc.dram_tensor("wqp", [D, HLOC * HD], F32R, kind="ExternalInput")
    wkp = nc.dram_tensor("wkp", [D, HLOC * HD], F32R, kind="ExternalInput")
    wvp = nc.dram_tensor("wvp", [D, HLOC * HD], F32R, kind="ExternalInput")
    wog = nc.dram_tensor("wog", [D, 256], BF16, kind="ExternalInput")
    cs1 = nc.dram_tensor("cs1", [128, S], F32R, kind="ExternalInput")
    cs2 = nc.dram_tensor("cs2", [128, S], F32R, kind="ExternalInput")
    mband = nc.dram_tensor("mband", [128, 4 * QC], F32, kind="ExternalInput")
    out = nc.dram_tensor("out", [256, T], F32, kind="ExternalOutput")

    xT_re = xT.ap().rearrange("(a p) t -> p a t", p=128)    # [128, 16, T]
    wq_re = wqp.ap().rearrange("(a p) c -> p a c", p=128)   # [128, 16, 256]
    wk_re = wkp.ap().rearrange("(a p) c -> p a c", p=128)
    wv_re = wvp.ap().rearrange("(a p) c -> p a c", p=128)
    wog_re = wog.ap().rearrange("(a p) c -> p a c", p=128)  # [128, 16, 256]

    with tile.TileContext(nc) as tc:
        with ExitStack() as stack:
            # ---- persistent small tiles ----
            const_pool = stack.enter_context(tc.tile_pool(name="const", bufs=1))
            cs1_sb = const_pool.tile([128, S], F32R, name="cs1_sb")
            cs2_sb = const_pool.tile([128, S], F32R, name="cs2_sb")
            mband_sb = const_pool.tile([128, 4, QC], F32, name="mband_sb")
            ones_f = const_pool.tile([128, 1], F32, name="ones_f")
            ones_col = const_pool.tile([128, 1], F32R, name="ones_col")
            onesr_f = const_pool.tile([1, 128], F32, name="onesr_f")
            ones_row = const_pool.tile([1, 128], F32R, name="ones_row")
            nc.sync.dma_start(cs1_sb[:], cs1.ap())
            nc.sync.dma_start(cs2_sb[:], cs2.ap())
            nc.sync.dma_start(
                mband_sb[:], mband.ap().rearrange("p (j q) -> p j q", j=4)
            )
            nc.vector.memset(ones_f[:], 1.0)
            nc.vector.tensor_copy(ones_col[:], ones_f[:])
            nc.vector.memset(onesr_f[:], 1.0)
            nc.vector.tensor_copy(ones_row[:], onesr_f[:])

            # ---- weights ----
            w_pool = stack.enter_context(tc.tile_pool(name="wpool", bufs=1))
            wq_sb = w_pool.tile([128, NDT, 256], F32R, name="wq_sb")
            wk_sb = w_pool.tile([128, NDT, 256], F32R, name="wk_sb")
            wv_sb = w_pool.tile([128, NDT, 256], F32R, name="wv_sb")
            nc.sync.dma_start(wq_sb[:], wq_re)
            nc.sync.dma_start(wk_sb[:], wk_re)
            nc.sync.dma_start(wv_sb[:], wv_re)

            # ---- per-batch qkv storage (reused across batches) ----
            qkv_pool = stack.enter_context(tc.tile_pool(name="qkv", bufs=1))
            q_sb = qkv_pool.tile([128, HLOC, S], F32R, name="q_sb")
            k_sb = qkv_pool.tile([128, HLOC, S], F32R, name="k_sb")
            v_sb = qkv_pool.tile([128, S // KT, HLOC * HD], F32R, name="v_sb")

            # ---- working pools ----
            x_pool = stack.enter_context(tc.tile_pool(name="xc", bufs=2))
            rope_pool = stack.enter_context(tc.tile_pool(name="rope", bufs=1))
            ps_qk = stack.enter_context(
                tc.tile_pool(name="psqk", bufs=2, space="PSUM")
            )
            ps_v = stack.enter_context(
                tc.tile_pool(name="psv", bufs=1, space="PSUM")
            )
            ps_sc = stack.enter_context(
                tc.tile_pool(name="pssc", bufs=2, space="PSUM")
            )
            ps_pv = stack.enter_context(
                tc.tile_pool(name="pspv", bufs=2, space="PSUM")
            )
            ps_lp = stack.enter_context(
                tc.tile_pool(name="pslp", bufs=1, space="PSUM")
            )
            pt_pool = stack.enter_context(tc.tile_pool(name="ptp", bufs=4))
            small_pool = stack.enter_context(tc.tile_pool(name="smallp", bufs=2))
            ao_pool = stack.enter_context(tc.tile_pool(name="aop", bufs=4))

            # ---- DRAM bounce buffers for the collectives ----
            dram_pool = stack.enter_context(
                tc.tile_pool(name="dram", bufs=1, space="DRAM")
            )
            ao_dram = [
                dram_pool.tile([HD, T], BF16, name=f"ao{h}_dram")
                for h in range(HLOC)
            ]
            g_dram = [
                dram_pool.tile(
                    [HD * NCORES, T], BF16, addr_space="Shared", name=f"g{h}_dram"
                )
                for h in range(HLOC)
            ]

            for b in range(B):
                tb = b * S  # global token offset of this batch

                # ======== QKV projection for batch b ========
                for ch in range(NCHK_B):
                    t0 = tb + ch * CHK
                    xc = x_pool.tile([128, NDT, CHK], F32R, tag="xc")
                    nc.sync.dma_start(xc[:], xT_re[:, :, t0 : t0 + CHK])
                    for h in range(HLOC):
                        psq = ps_qk.tile([128, CHK], F32, tag="psqk")
                        psk = ps_qk.tile([128, CHK], F32, tag="psqk")
                        for dt in range(NDT):
                            nc.tensor.matmul(
                                psq[:],
                                lhsT=wq_sb[:, dt, h * HD : (h + 1) * HD],
                                rhs=xc[:, dt, :],
                                start=(dt == 0),
                                stop=(dt == NDT - 1),
                            )
                        for dt in range(NDT):
                            nc.tensor.matmul(
                                psk[:],
                                lhsT=wk_sb[:, dt, h * HD : (h + 1) * HD],
                                rhs=xc[:, dt, :],
                                start=(dt == 0),
                                stop=(dt == NDT - 1),
                            )
                        nc.vector.tensor_copy(
                            q_sb[:, h, ch * CHK : (ch + 1) * CHK], psq[:]
                        )
                        nc.vector.tensor_copy(
                            k_sb[:, h, ch * CHK : (ch + 1) * CHK], psk[:]
                        )
                    for st in range(CHK // KT):
                        psv = ps_v.tile([128, HLOC * HD], F32, tag="psv")
                        for dt in range(NDT):
                            nc.tensor.matmul(
                                psv[:],
                                lhsT=xc[:, dt, st * KT : (st + 1) * KT],
                                rhs=wv_sb[:, dt, :],
                                start=(dt == 0),
                                stop=(dt == NDT - 1),
                            )
                        nc.vector.tensor_copy(
                            v_sb[:, ch * (CHK // KT) + st, :], psv[:]
                        )

                # ======== RoPE for batch b (in place on q_sb/k_sb) ========
                # layout per head: partitions 0:64 = even pair elems (t0),
                # 64:128 = odd (t1).  cs1 = [cos; -sin], cs2 = [sin; cos].
                SH = S // 2
                for tens in (q_sb, k_sb):
                    for h in range(HLOC):
                        for hf in range(2):
                            sl = slice(hf * SH, (hf + 1) * SH)
                            t1 = rope_pool.tile([128, SH], F32R, tag="t1")
                            t2 = rope_pool.tile([128, SH], F32R, tag="t2")
                            t1s = rope_pool.tile([64, SH], F32R, tag="t1s")
                            t2s = rope_pool.tile([64, SH], F32R, tag="t2s")
                            src = tens[:, h, sl]
                            nc.vector.tensor_mul(t1[:], src, cs1_sb[:, sl])
                            nc.vector.tensor_mul(t2[:], src, cs2_sb[:, sl])
                            # shift odd-half products to partitions 0:64 (ACT)
                            nc.scalar.copy(t1s[:], t1[64:128, :])
                            nc.scalar.copy(t2s[:], t2[64:128, :])
                            nc.vector.tensor_add(
                                tens[0:64, h, sl], t1[0:64, :], t1s[:]
                            )
                            nc.vector.tensor_add(
                                tens[64:128, h, sl], t2[0:64, :], t2s[:]
                            )

                # ======== attention for batch b ========
                for h in range(HLOC):
                    for tcq in range(NQC_B):
                        q0 = tcq * QC
                        nkt = (tcq + 1) * (QC // KT)
                        acc = small_pool.tile([128, QC], F32R, tag="acc")
                        pv = ps_pv.tile([128, QC], F32, tag="pv")
                        for kt in range(nkt):
                            k0 = kt * KT
                            ps = ps_sc.tile([128, QC], F32, tag="sc")
                            nc.tensor.matmul(
                                ps[:],
                                lhsT=k_sb[:, h, k0 : k0 + KT],
                                rhs=q_sb[:, h, q0 : q0 + QC],
                                start=True,
                                stop=True,
                            )
                            j = kt - (QC // KT) * tcq
                            if j >= 0:
                                nc.vector.tensor_add(
                                    ps[:], ps[:], mband_sb[:, j, :]
                                )
                            pt = pt_pool.tile([128, QC], F32R, tag="pt")
                            nc.scalar.activation(pt[:], ps[:], EXP, scale=SCALE)
                            if kt == 0:
                                nc.vector.tensor_copy(acc[:], pt[:])
                            else:
                                nc.vector.tensor_add(acc[:], acc[:], pt[:])
                            nc.tensor.matmul(
                                pv[:],
                                lhsT=v_sb[:, kt, h * HD : (h + 1) * HD],
                                rhs=pt[:],
                                start=(kt == 0),
                                stop=(kt == nkt - 1),
                            )
                        # l = column sums of acc; recip; broadcast; normalize
                        lp = ps_lp.tile([1, QC], F32, tag="lp")
                        nc.tensor.matmul(
                            lp[:],
                            lhsT=ones_col[:],
                            rhs=acc[:],
                            start=True,
                            stop=True,
                        )
                        rec = small_pool.tile([1, QC], F32, tag="rec")
                        nc.vector.reciprocal(rec[:], lp[:])
                        rec_r = small_pool.tile([1, QC], F32R, tag="rec_r")
                        nc.vector.tensor_copy(rec_r[:], rec[:])
                        rb = ps_sc.tile([128, QC], F32, tag="sc")
                        nc.tensor.matmul(
                            rb[:],
                            lhsT=ones_row[:],
                            rhs=rec_r[:],
                            start=True,
                            stop=True,
                        )
                        rbs = small_pool.tile([128, QC], F32, tag="rbs")
                        nc.vector.tensor_copy(rbs[:], rb[:])
                        aon = ao_pool.tile([128, QC], BF16, tag="aon")
                        nc.vector.tensor_mul(aon[:], pv[:], rbs[:])
                        nc.sync.dma_start(
                            ao_dram[h][:, tb + q0 : tb + q0 + QC], aon[:]
                        )
                    if b == B - 1:
                        # all of head-slot h's output is now written
                        nc.gpsimd.collective_compute(
                            "AllGather",
                            mybir.AluOpType.bypass,
                            replica_groups=[list(range(NCORES))],
                            ins=[ao_dram[h].opt()],
                            outs=[g_dram[h].opt()],
                        )

        # ======== stage 2: out_T[od, tok] = sum_ad woT_g[ad, od] * AO_T[ad, tok]
        with ExitStack() as stack2:
            s2_pool = stack2.enter_context(tc.tile_pool(name="s2", bufs=1))
            g_pool = stack2.enter_context(tc.tile_pool(name="gp", bufs=3))
            s2_psum = stack2.enter_context(
                tc.tile_pool(name="s2ps", bufs=1, space="PSUM")
            )
            wog_sb = s2_pool.tile([128, NDT, 256], BF16, name="wog_sb")
            nc.sync.dma_start(wog_sb[:], wog_re)
            NTC = T // QC  # 8 token chunks
            for od in range(2):
                psos = [
                    s2_psum.tile([128, QC], F32, tag=f"so{i}", name=f"so{i}_{od}")
                    for i in range(NTC)
                ]
                for ad in range(NDT):
                    gsrc = g_dram[ad // NCORES]
                    row0 = (ad % NCORES) * 128
                    gt = g_pool.tile([128, T], BF16, tag="gt")
                    nc.sync.dma_start(gt[:], gsrc[row0 : row0 + 128, :])
                    for tcg in range(NTC):
                        nc.tensor.matmul(
                            psos[tcg][:],
                            lhsT=wog_sb[:, ad, od * 128 : (od + 1) * 128],
                            rhs=gt[:, tcg * QC : (tcg + 1) * QC],
                            start=(ad == 0),
                            stop=(ad == NDT - 1),
                        )
                for tcg in range(NTC):
                    ost = g_pool.tile([128, QC], F32, tag="ost", name=f"ost{od}_{tcg}")
                    nc.vector.tensor_copy(ost[:], psos[tcg][:])
                    nc.sync.dma_start(
                        out.ap()[od * 128 : (od + 1) * 128, tcg * QC : (tcg + 1) * QC],
                        ost[:],
                    )

    nc.compile()
    return nc


_CACHED = {}


def _get_compiled():
    if "nc" not in _CACHED:
        _CACHED["nc"] = build_kernel()
    return _CACHED["nc"]


def shard_inputs(x, wq, wk, wv, wo, freqs_cos, freqs_sin, mask):
    x = np.asarray(x, np.float32)
    wq = np.asarray(wq, np.float32)
    wk = np.asarray(wk, np.float32)
    wv = np.asarray(wv, np.float32)
    wo = np.asarray(wo, np.float32)
    fc = np.asarray(freqs_cos, np.float32)
    fs = np.asarray(freqs_sin, np.float32)
    mask = np.asarray(mask, np.float32)

    xT = np.ascontiguousarray(x.reshape(T, D).T)  # [D, T]

    # de-interleave within each head: [0,2,...,126, 1,3,...,127]
    perm = np.concatenate([np.arange(0, HD, 2), np.arange(1, HD, 2)])

    ct = fc.T  # [64, S]
    st = fs.T
    cs1 = np.ascontiguousarray(np.concatenate([ct, -st], axis=0))  # [128, S]
    cs2 = np.ascontiguousarray(np.concatenate([st, ct], axis=0))

    # mask bands: band j = sqrt(HD) * mask[0,0, 0:QC, 128j:128(j+1)].T
    m = mask[0, 0]
    mb = np.concatenate(
        [SQHD * m[0:QC, KT * j : KT * (j + 1)].T for j in range(QC // KT)], axis=1
    ).astype(np.float32)  # [128, 4*QC]
    mb = np.ascontiguousarray(mb)

    # wo: gathered row order is [even heads, odd heads] (slot-major)
    woT = wo.T  # [D(ad), D(od)]
    head_order = list(range(0, H, 2)) + list(range(1, H, 2))
    woT_g = woT.reshape(H, HD, D)[head_order].reshape(D, D)

    in_maps = []
    for c in range(NCORES):
        rows = slice(c * HLOC * HD, (c + 1) * HLOC * HD)
        wq_c = wq[rows].reshape(HLOC, HD, D)[:, perm, :].reshape(HLOC * HD, D)
        wk_c = wk[rows].reshape(HLOC, HD, D)[:, perm, :].reshape(HLOC * HD, D)
        in_maps.append(
            {
                "xT": xT,
                "wqp": np.ascontiguousarray(wq_c.T),
                "wkp": np.ascontiguousarray(wk_c.T),
                "wvp": np.ascontiguousarray(wv[rows].T),
                "wog": np.ascontiguousarray(
                    woT_g[:, c * 256 : (c + 1) * 256]
                ).astype(NP_BF16),
                "cs1": cs1,
                "cs2": cs2,
                "mband": mb,
            }
        )
    return in_maps


def run_sharded(in_maps, trace=False):
    nc = _get_compiled()
    res = bass_utils.run_bass_kernel_spmd(
        nc, in_maps, core_ids=list(range(NCORES)), trace=trace
    )
    return res


def unshard(results):
    # results: list of dicts with "out": [256, T]
    out_T = np.concatenate([r["out"] for r in results], axis=0)  # [D, T]
    return np.ascontiguousarray(out_T.T).reshape(B, S, D)


def kernel(**inputs):
    in_maps = shard_inputs(**inputs)
    res = run_sharded(in_maps, trace=False)
    return unshard(res.results)


# revision 14
# speedup vs baseline: 1.2606x; 1.2606x over previous
"""Distributed Bass kernel for nn_Attention_6287832122083 on 8 TRN2 NeuronCores.

Strategy: tensor-parallel over heads (2 heads per core).
 - Each core computes q,k,v for its 2 heads (f32r matmuls), applies RoPE,
   runs causal attention (skipping fully-masked key blocks), normalizes,
   and AllGathers the per-head attention outputs (bf16, one collective per
   (batch, head-slot) so stage 2 overlaps compute).
 - Each core then computes a 256-column slice of the final output through
   wo, producing an output-transposed [256, B*S] slice.
 - Host unshard = concatenate slices + transpose (no arithmetic).
"""

import math
from contextlib import ExitStack

import numpy as np

import concourse.bass as bass
import concourse.bacc as bacc
import concourse.mybir as mybir
import concourse.tile as tile
from concourse import bass_utils

F32 = mybir.dt.float32
F32R = mybir.dt.float32r
BF16 = mybir.dt.bfloat16
NP_BF16 = mybir.dt.np(BF16)
EXP = mybir.ActivationFunctionType.Exp

B, S, D, H = 2, 2048, 2048, 16
HD = D // H              # 128
T = B * S                # 4096 tokens
NCORES = 8
HLOC = H // NCORES       # 2 heads per core
CHK = 256                # QKV token chunk
NCHK_B = S // CHK        # 8 chunks per batch
QC = 512                 # attention query chunk
NQC_B = S // QC          # 4 query chunks per batch
KT = 128                 # key tile
SCALE = 1.0 / math.sqrt(HD)
SQHD = math.sqrt(HD)
NDT = D // 128           # 16 contraction tiles


def build_kernel():
    nc = bacc.Bacc(
        "TRN2",
        target_bir_lowering=False,
        debug=False,
        enable_asserts=False,
        num_devices=NCORES,
    )

    # Per-core DRAM parameters (f32r tensors carry plain fp32 bits).
    xT = nc.dram_tensor("xT", [D, T], F32R, kind="ExternalInput")
    wqp = nc.dram_tensor("wqp", [D, HLOC * HD], F32R, kind="ExternalInput")
    wkp = nc.dram_tensor("wkp", [D, HLOC * HD], F32R, kind="ExternalInput")
    wvp = nc.dram_tensor("wvp", [D, HLOC * HD], F32R, kind="ExternalInput")
    wog = nc.dram_tensor("wog", [D, 256], BF16, kind="ExternalInput")
    cs1 = nc.dram_tensor("cs1", [128, S], F32R, kind="ExternalInput")
    cs2 = nc.dram_tensor("cs2", [128, S], F32R, kind="ExternalInput")
    mband = nc.dram_tensor("mband", [128, 4 * QC], F32R, kind="ExternalInput")
    ident = nc.dram_tensor("ident", [128, 128], F32R, kind="ExternalInput")
    out = nc.dram_tensor("out", [256, T], F32, kind="ExternalOutput")

    xT_re = xT.ap().rearrange("(a p) t -> p a t", p=128)    # [128, 16, T]
    wq_re = wqp.ap().rearrange("(a p) c -> p a c", p=128)   # [128, 16, 256]
    wk_re = wkp.ap().rearrange("(a p) c -> p a c", p=128)
    wv_re = wvp.ap().rearrange("(a p) c -> p a c", p=128)
    wog_re = wog.ap().rearrange("(a p) c -> p a c", p=128)  # [128, 16, 256]

    with tile.TileContext(nc) as tc:
        with ExitStack() as stack:
            # ---- persistent small tiles ----
            const_pool = stack.enter_context(tc.tile_pool(name="const", bufs=1))
            cs1_sb = const_pool.tile([128, S], F32R, name="cs1_sb")
            cs2_sb = const_pool.tile([128, S], F32R, name="cs2_sb")
            mband_sb = const_pool.tile([128, 4, QC], F32R, name="mband_sb")
            id_sb = const_pool.tile([128, 128], F32R, name="id_sb")
            ones_f = const_pool.tile([128, 128], F32, name="ones_f")
            ones128 = const_pool.tile([128, 128], F32R, name="ones128")
            nc.sync.dma_start(cs1_sb[:], cs1.ap())
            nc.sync.dma_start(cs2_sb[:], cs2.ap())
            nc.sync.dma_start(
                mband_sb[:], mband.ap().rearrange("p (j q) -> p j q", j=4)
            )
            nc.sync.dma_start(id_sb[:], ident.ap())
            nc.vector.memset(ones_f[:], 1.0)
            nc.vector.tensor_copy(ones128[:], ones_f[:])

            # ---- weights ----
            w_pool = stack.enter_context(tc.tile_pool(name="wpool", bufs=1))
            wq_sb = w_pool.tile([128, NDT, 256], F32R, name="wq_sb")
            wk_sb = w_pool.tile([128, NDT, 256], F32R, name="wk_sb")
            wv_sb = w_pool.tile([128, NDT, 256], F32R, name="wv_sb")
            nc.sync.dma_start(wq_sb[:], wq_re)
            nc.sync.dma_start(wk_sb[:], wk_re)
            nc.sync.dma_start(wv_sb[:], wv_re)

            # ---- per-batch qkv storage (reused across batches) ----
            qkv_pool = stack.enter_context(tc.tile_pool(name="qkv", bufs=1))
            q_sb = qkv_pool.tile([128, HLOC, S], F32R, name="q_sb")
            k_sb = qkv_pool.tile([128, HLOC, S], F32R, name="k_sb")
            v_sb = qkv_pool.tile([128, S // KT, HLOC * HD], F32R, name="v_sb")

            # ---- working pools ----
            ps_sc = stack.enter_context(
                tc.tile_pool(name="pssc", bufs=3, space="PSUM")
            )
            ps_pv = stack.enter_context(
                tc.tile_pool(name="pspv", bufs=2, space="PSUM")
            )
            pt_pool = stack.enter_context(tc.tile_pool(name="ptp", bufs=4))
            small_pool = stack.enter_context(tc.tile_pool(name="smallp", bufs=2))
            ao_pool = stack.enter_context(tc.tile_pool(name="aop", bufs=4))

            # ---- DRAM bounce buffers for the collectives ----
            dram_pool = stack.enter_context(
                tc.tile_pool(name="dram", bufs=1, space="DRAM")
            )
            ao_dram = [
                [
                    dram_pool.tile([HD, S], BF16, name=f"ao{h}_{b}_dram")
                    for b in range(B)
                ]
                for h in range(HLOC)
            ]
            g_dram = [
                [
                    dram_pool.tile(
                        [HD * NCORES, S],
                        BF16,
                        addr_space="Shared",
                        name=f"g{h}_{b}_dram",
                    )
                    for b in range(B)
                ]
                for h in range(HLOC)
            ]

            # QKV-only pools opened last so they can be popped early (LIFO),
            # freeing SBUF + PSUM for stage 2 while attention(b1) runs.
            qkv_ps_stack = ExitStack()
            x_pool = qkv_ps_stack.enter_context(tc.tile_pool(name="xc", bufs=2))
            rope_pool = qkv_ps_stack.enter_context(
                tc.tile_pool(name="rope", bufs=1)
            )
            ps_qk = qkv_ps_stack.enter_context(
                tc.tile_pool(name="psqk", bufs=2, space="PSUM")
            )
            ps_v = qkv_ps_stack.enter_context(
                tc.tile_pool(name="psv", bufs=1, space="PSUM")
            )

            SH = S // 2

            def emit_rope(tens, hf):
                # RoPE halves: partitions 0:64 even pair elems, 64:128 odd.
                # cs1 = [cos; -sin], cs2 = [sin; cos].
                for h in range(HLOC):
                    sl = slice(hf * SH, (hf + 1) * SH)
                    t1 = rope_pool.tile([128, SH], F32R, tag="t1")
                    t2 = rope_pool.tile([128, SH], F32R, tag="t2")
                    t1s = rope_pool.tile([64, SH], F32R, tag="t1s")
                    t2s = rope_pool.tile([64, SH], F32R, tag="t2s")
                    src = tens[:, h, sl]
                    nc.vector.tensor_mul(t1[:], src, cs1_sb[:, sl])
                    nc.vector.tensor_mul(t2[:], src, cs2_sb[:, sl])
                    nc.scalar.copy(t1s[:], t1[64:128, :])
                    nc.scalar.copy(t2s[:], t2[64:128, :])
                    nc.vector.tensor_add(tens[0:64, h, sl], t1[0:64, :], t1s[:])
                    nc.vector.tensor_add(
                        tens[64:128, h, sl], t2[0:64, :], t2s[:]
                    )

            def emit_qkv(b):
                tb = b * S
                for ch in range(NCHK_B):
                    t0 = tb + ch * CHK
                    xc = x_pool.tile([128, NDT, CHK], F32R, tag="xc")
                    nc.sync.dma_start(xc[:], xT_re[:, :, t0 : t0 + CHK])
                    for h in range(HLOC):
                        psq = ps_qk.tile([128, CHK], F32, tag="psqk")
                        psk = ps_qk.tile([128, CHK], F32, tag="psqk")
                        for dt in range(NDT):
                            nc.tensor.matmul(
                                psq[:],
                                lhsT=wq_sb[:, dt, h * HD : (h + 1) * HD],
                                rhs=xc[:, dt, :],
                                start=(dt == 0),
                                stop=(dt == NDT - 1),
                            )
                        for dt in range(NDT):
                            nc.tensor.matmul(
                                psk[:],
                                lhsT=wk_sb[:, dt, h * HD : (h + 1) * HD],
                                rhs=xc[:, dt, :],
                                start=(dt == 0),
                                stop=(dt == NDT - 1),
                            )
                        nc.vector.tensor_copy(
                            q_sb[:, h, ch * CHK : (ch + 1) * CHK], psq[:]
                        )
                        nc.vector.tensor_copy(
                            k_sb[:, h, ch * CHK : (ch + 1) * CHK], psk[:]
                        )
                    for st in range(CHK // KT):
                        psv = ps_v.tile([128, HLOC * HD], F32, tag="psv")
                        for dt in range(NDT):
                            nc.tensor.matmul(
                                psv[:],
                                lhsT=xc[:, dt, st * KT : (st + 1) * KT],
                                rhs=wv_sb[:, dt, :],
                                start=(dt == 0),
                                stop=(dt == NDT - 1),
                            )
                        nc.vector.tensor_copy(
                            v_sb[:, ch * (CHK // KT) + st, :], psv[:]
                        )
                    if ch == NCHK_B // 2 - 1:
                        for tens in (q_sb, k_sb):
                            emit_rope(tens, 0)
                    elif ch == NCHK_B - 1:
                        for tens in (q_sb, k_sb):
                            emit_rope(tens, 1)

            def emit_attention(b, h):
                tb = b * S
                for tcq in range(NQC_B):
                    q0 = tcq * QC
                    nkt = (tcq + 1) * (QC // KT)
                    acc = small_pool.tile([128, QC], F32R, tag="acc")
                    pv = ps_pv.tile([128, QC], F32, tag="pv")
                    for kt in range(nkt):
                        k0 = kt * KT
                        j = kt - (QC // KT) * tcq
                        ps = ps_sc.tile([128, QC], F32, tag="sc")
                        if j >= 0:
                            # preload additive mask band into PSUM, then
                            # accumulate the scores matmul on top of it
                            nc.tensor.matmul(
                                ps[:],
                                lhsT=id_sb[:],
                                rhs=mband_sb[:, j, :],
                                start=True,
                                stop=False,
                            )
                        nc.tensor.matmul(
                            ps[:],
                            lhsT=k_sb[:, h, k0 : k0 + KT],
                            rhs=q_sb[:, h, q0 : q0 + QC],
                            start=(j < 0),
                            stop=True,
                        )
                        pt = pt_pool.tile([128, QC], F32R, tag="pt")
                        nc.scalar.activation(pt[:], ps[:], EXP, scale=SCALE)
                        if kt == 0:
                            nc.vector.tensor_copy(acc[:], pt[:])
                        else:
                            nc.vector.tensor_add(acc[:], acc[:], pt[:])
                        nc.tensor.matmul(
                            pv[:],
                            lhsT=v_sb[:, kt, h * HD : (h + 1) * HD],
                            rhs=pt[:],
                            start=(kt == 0),
                            stop=(kt == nkt - 1),
                        )
                    # broadcast row-sums via all-ones matmul: every output
                    # partition gets l[tq]; then 128-lane reciprocal.
                    lb = ps_sc.tile([128, QC], F32, tag="sc")
                    nc.tensor.matmul(
                        lb[:], lhsT=ones128[:], rhs=acc[:], start=True, stop=True
                    )
                    rbs = small_pool.tile([128, QC], F32, tag="rbs")
                    nc.vector.reciprocal(rbs[:], lb[:])
                    aon = ao_pool.tile([128, QC], BF16, tag="aon")
                    nc.vector.tensor_mul(aon[:], pv[:], rbs[:])
                    nc.sync.dma_start(ao_dram[h][b][:, q0 : q0 + QC], aon[:])
                # head-slot h, batch b fully written -> gather it
                nc.gpsimd.collective_compute(
                    "AllGather",
                    mybir.AluOpType.bypass,
                    replica_groups=[list(range(NCORES))],
                    ins=[ao_dram[h][b].opt()],
                    outs=[g_dram[h][b].opt()],
                )

            # ======== stage 2 helpers ========
            s2_open = {}

            def emit_stage2(b, s2_pools):
                s2_pool, g_pool, s2_psum = s2_pools
                wog_sb = s2_open.get("wog_sb")
                if wog_sb is None:
                    wog_sb = s2_pool.tile([128, NDT, 256], BF16, name="wog_sb")
                    nc.sync.dma_start(wog_sb[:], wog_re)
                    s2_open["wog_sb"] = wog_sb
                for tcg in range(NQC_B):
                    gts = []
                    for ad in range(NDT):
                        gt = g_pool.tile(
                            [128, QC],
                            BF16,
                            tag="gt",
                            bufs=18,
                            name=f"gt{b}_{tcg}_{ad}",
                        )
                        gsrc = g_dram[ad // NCORES][b]
                        row0 = (ad % NCORES) * 128
                        nc.sync.dma_start(
                            gt[:],
                            gsrc[row0 : row0 + 128, tcg * QC : (tcg + 1) * QC],
                        )
                        gts.append(gt)
                    for od in range(2):
                        pso = s2_psum.tile(
                            [128, QC], F32, tag="so", name=f"so{b}_{tcg}_{od}"
                        )
                        for ad in range(NDT):
                            nc.tensor.matmul(
                                pso[:],
                                lhsT=wog_sb[:, ad, od * 128 : (od + 1) * 128],
                                rhs=gts[ad][:],
                                start=(ad == 0),
                                stop=(ad == NDT - 1),
                            )
                        ost = g_pool.tile(
                            [128, QC], F32, tag="ost", name=f"ost{b}_{tcg}_{od}"
                        )
                        nc.vector.tensor_copy(ost[:], pso[:])
                        nc.sync.dma_start(
                            out.ap()[
                                od * 128 : (od + 1) * 128,
                                b * S + tcg * QC : b * S + (tcg + 1) * QC,
                            ],
                            ost[:],
                        )

            # ======== emission schedule ========
            emit_qkv(0)
            for h in range(HLOC):
                emit_attention(0, h)
            emit_qkv(1)
            # QKV psum pools no longer needed; free banks for stage 2
            qkv_ps_stack.close()
            s2_pool = stack.enter_context(tc.tile_pool(name="s2", bufs=1))
            g_pool = stack.enter_context(tc.tile_pool(name="gp", bufs=2))
            s2_psum = stack.enter_context(
                tc.tile_pool(name="s2ps", bufs=2, space="PSUM")
            )
            s2_pools = (s2_pool, g_pool, s2_psum)
            emit_stage2(0, s2_pools)  # overlaps batch-1 attention
            for h in range(HLOC):
                emit_attention(1, h)
            emit_stage2(1, s2_pools)

    nc.compile()
    return nc


_CACHED = {}


def _get_compiled():
    if "nc" not in _CACHED:
        _CACHED["nc"] = build_kernel()
    return _CACHED["nc"]


def shard_inputs(x, wq, wk, wv, wo, freqs_cos, freqs_sin, mask):
    x = np.asarray(x, np.float32)
    wq = np.asarray(wq, np.float32)
    wk = np.asarray(wk, np.float32)
    wv = np.asarray(wv, np.float32)
    wo = np.asarray(wo, np.float32)
    fc = np.asarray(freqs_cos, np.float32)
    fs = np.asarray(freqs_sin, np.float32)
    mask = np.asarray(mask, np.float32)

    xT = np.ascontiguousarray(x.reshape(T, D).T)  # [D, T]

    # de-interleave within each head: [0,2,...,126, 1,3,...,127]
    perm = np.concatenate([np.arange(0, HD, 2), np.arange(1, HD, 2)])

    ct = fc.T  # [64, S]
    st = fs.T
    cs1 = np.ascontiguousarray(np.concatenate([ct, -st], axis=0))  # [128, S]
    cs2 = np.ascontiguousarray(np.concatenate([st, ct], axis=0))

    # mask bands: band j = sqrt(HD) * mask[0,0, 0:QC, 128j:128(j+1)].T
    m = mask[0, 0]
    mb = np.concatenate(
        [SQHD * m[0:QC, KT * j : KT * (j + 1)].T for j in range(QC // KT)], axis=1
    ).astype(np.float32)  # [128, 4*QC]
    mb = np.ascontiguousarray(mb)

    identity = np.eye(128, dtype=np.float32)

    # wo: gathered row order is [even heads, odd heads] (slot-major)
    woT = wo.T  # [D(ad), D(od)]
    head_order = list(range(0, H, 2)) + list(range(1, H, 2))
    woT_g = woT.reshape(H, HD, D)[head_order].reshape(D, D)

    in_maps = []
    for c in range(NCORES):
        rows = slice(c * HLOC * HD, (c + 1) * HLOC * HD)
        wq_c = wq[rows].reshape(HLOC, HD, D)[:, perm, :].reshape(HLOC * HD, D)
        wk_c = wk[rows].reshape(HLOC, HD, D)[:, perm, :].reshape(HLOC * HD, D)
        in_maps.append(
            {
                "xT": xT,
                "wqp": np.ascontiguousarray(wq_c.T),
                "wkp": np.ascontiguousarray(wk_c.T),
                "wvp": np.ascontiguousarray(wv[rows].T),
                "wog": np.ascontiguousarray(
                    woT_g[:, c * 256 : (c + 1) * 256]
                ).astype(NP_BF16),
                "cs1": cs1,
                "cs2": cs2,
                "mband": mb,
                "ident": identity,
            }
        )
    return in_maps


def run_sharded(in_maps, trace=False):
    nc = _get_compiled()
    res = bass_utils.run_bass_kernel_spmd(
        nc, in_maps, core_ids=list(range(NCORES)), trace=trace
    )
    return res


def unshard(results):
    # results: list of dicts with "out": [256, T]
    out_T = np.concatenate([r["out"] for r in results], axis=0)  # [D, T]
    return np.ascontiguousarray(out_T.T).reshape(B, S, D)


def kernel(**inputs):
    in_maps = shard_inputs(**inputs)
    res = run_sharded(in_maps, trace=False)
    return unshard(res.results)


# revision 15
# speedup vs baseline: 1.4358x; 1.1390x over previous
"""Distributed Bass kernel for nn_Attention_6287832122083 on 8 TRN2 NeuronCores.

Strategy: tensor-parallel over heads (2 heads per core).
 - Each core computes q,k,v for its 2 heads (f32r matmuls), applies RoPE,
   runs causal attention (skipping fully-masked key blocks), normalizes,
   and AllGathers the per-head attention outputs (bf16, one collective per
   (batch, head-slot) so stage 2 overlaps compute).
 - Each core then computes a 256-column slice of the final output through
   wo, producing an output-transposed [256, B*S] slice.
 - Host unshard = concatenate slices + transpose (no arithmetic).
"""

import math
from contextlib import ExitStack

import numpy as np

import concourse.bass as bass
import concourse.bacc as bacc
import concourse.mybir as mybir
import concourse.tile as tile
from concourse import bass_utils

F32 = mybir.dt.float32
F32R = mybir.dt.float32r
BF16 = mybir.dt.bfloat16
NP_BF16 = mybir.dt.np(BF16)
EXP = mybir.ActivationFunctionType.Exp

B, S, D, H = 2, 2048, 2048, 16
HD = D // H              # 128
T = B * S                # 4096 tokens
NCORES = 8
HLOC = H // NCORES       # 2 heads per core
CHK = 256                # QKV token chunk
NCHK_B = S // CHK        # 8 chunks per batch
QC = 512                 # attention query chunk
NQC_B = S // QC          # 4 query chunks per batch
KT = 128                 # key tile
SCALE = 1.0 / math.sqrt(HD)
SQHD = math.sqrt(HD)
NDT = D // 128           # 16 contraction tiles


def build_kernel():
    nc = bacc.Bacc(
        "TRN2",
        target_bir_lowering=False,
        debug=False,
        enable_asserts=False,
        num_devices=NCORES,
    )

    # Per-core DRAM parameters (f32r tensors carry plain fp32 bits).
    xT = nc.dram_tensor("xT", [D, T], F32R, kind="ExternalInput")
    wqp = nc.dram_tensor("wqp", [D, HLOC * HD], F32R, kind="ExternalInput")
    wkp = nc.dram_tensor("wkp", [D, HLOC * HD], F32R, kind="ExternalInput")
    wvp = nc.dram_tensor("wvp", [D, HLOC * HD], F32R, kind="ExternalInput")
    wog = nc.dram_tensor("wog", [D, 256], BF16, kind="ExternalInput")
    cs1 = nc.dram_tensor("cs1", [128, S], F32R, kind="ExternalInput")
    cs2 = nc.dram_tensor("cs2", [128, S], F32R, kind="ExternalInput")
    mband = nc.dram_tensor("mband", [128, 4 * QC], F32R, kind="ExternalInput")
    ident = nc.dram_tensor("ident", [128, 128], F32R, kind="ExternalInput")
    out = nc.dram_tensor("out", [256, T], F32, kind="ExternalOutput")

    xT_re = xT.ap().rearrange("(a p) t -> p a t", p=128)    # [128, 16, T]
    wq_re = wqp.ap().rearrange("(a p) c -> p a c", p=128)   # [128, 16, 256]
    wk_re = wkp.ap().rearrange("(a p) c -> p a c", p=128)
    wv_re = wvp.ap().rearrange("(a p) c -> p a c", p=128)
    wog_re = wog.ap().rearrange("(a p) c -> p a c", p=128)  # [128, 16, 256]

    with tile.TileContext(nc) as tc:
        with ExitStack() as stack:
            # ---- persistent small tiles ----
            const_pool = stack.enter_context(tc.tile_pool(name="const", bufs=1))
            cs1_sb = const_pool.tile([128, S], F32R, name="cs1_sb")
            cs2_sb = const_pool.tile([128, S], F32R, name="cs2_sb")
            mband_sb = const_pool.tile([128, 4, QC], F32R, name="mband_sb")
            id_sb = const_pool.tile([128, 128], F32R, name="id_sb")
            ones_f = const_pool.tile([128, 128], F32, name="ones_f")
            ones128 = const_pool.tile([128, 128], F32R, name="ones128")
            nc.sync.dma_start(cs1_sb[:], cs1.ap())
            nc.sync.dma_start(cs2_sb[:], cs2.ap())
            nc.sync.dma_start(
                mband_sb[:], mband.ap().rearrange("p (j q) -> p j q", j=4)
            )
            nc.sync.dma_start(id_sb[:], ident.ap())
            nc.vector.memset(ones_f[:], 1.0)
            nc.vector.tensor_copy(ones128[:], ones_f[:])

            # ---- weights ----
            w_pool = stack.enter_context(tc.tile_pool(name="wpool", bufs=1))
            wq_sb = w_pool.tile([128, NDT, 256], F32R, name="wq_sb")
            wk_sb = w_pool.tile([128, NDT, 256], F32R, name="wk_sb")
            wv_sb = w_pool.tile([128, NDT, 256], F32R, name="wv_sb")
            for i in range(4):
                dsl = slice(i * (NDT // 4), (i + 1) * (NDT // 4))
                nc.sync.dma_start(wq_sb[:, dsl, :], wq_re[:, dsl, :])
                nc.sync.dma_start(wk_sb[:, dsl, :], wk_re[:, dsl, :])
                nc.sync.dma_start(wv_sb[:, dsl, :], wv_re[:, dsl, :])

            # ---- per-batch qkv storage (reused across batches) ----
            qkv_pool = stack.enter_context(tc.tile_pool(name="qkv", bufs=1))
            q_sb = qkv_pool.tile([128, HLOC, S], F32R, name="q_sb")
            k_sb = qkv_pool.tile([128, HLOC, S], F32R, name="k_sb")
            v_sb = qkv_pool.tile([128, S // KT, HLOC * HD], F32R, name="v_sb")

            # ---- working pools ----
            ps_sc = stack.enter_context(
                tc.tile_pool(name="pssc", bufs=3, space="PSUM")
            )
            ps_pv = stack.enter_context(
                tc.tile_pool(name="pspv", bufs=2, space="PSUM")
            )
            pt_pool = stack.enter_context(tc.tile_pool(name="ptp", bufs=4))
            small_pool = stack.enter_context(tc.tile_pool(name="smallp", bufs=2))
            ao_pool = stack.enter_context(tc.tile_pool(name="aop", bufs=4))

            # ---- DRAM bounce buffers for the collectives ----
            dram_pool = stack.enter_context(
                tc.tile_pool(name="dram", bufs=1, space="DRAM")
            )
            ao_dram = [
                [
                    dram_pool.tile([HD, S], BF16, name=f"ao{h}_{b}_dram")
                    for b in range(B)
                ]
                for h in range(HLOC)
            ]
            g_dram = [
                [
                    dram_pool.tile(
                        [HD * NCORES, S],
                        BF16,
                        addr_space="Shared",
                        name=f"g{h}_{b}_dram",
                    )
                    for b in range(B)
                ]
                for h in range(HLOC)
            ]

            # QKV-only pools opened last so they can be popped early (LIFO),
            # freeing SBUF + PSUM for stage 2 while attention(b1) runs.
            qkv_ps_stack = ExitStack()
            x_pool = qkv_ps_stack.enter_context(tc.tile_pool(name="xc", bufs=3))
            rope_pool = qkv_ps_stack.enter_context(
                tc.tile_pool(name="rope", bufs=1)
            )
            ps_qk = qkv_ps_stack.enter_context(
                tc.tile_pool(name="psqk", bufs=2, space="PSUM")
            )
            ps_v = qkv_ps_stack.enter_context(
                tc.tile_pool(name="psv", bufs=1, space="PSUM")
            )

            SH = S // 2

            def emit_rope(tens, hf):
                # RoPE halves: partitions 0:64 even pair elems, 64:128 odd.
                # cs1 = [cos; -sin], cs2 = [sin; cos].
                for h in range(HLOC):
                    sl = slice(hf * SH, (hf + 1) * SH)
                    t1 = rope_pool.tile([128, SH], F32R, tag="t1")
                    t2 = rope_pool.tile([128, SH], F32R, tag="t2")
                    t1s = rope_pool.tile([64, SH], F32R, tag="t1s")
                    t2s = rope_pool.tile([64, SH], F32R, tag="t2s")
                    src = tens[:, h, sl]
                    nc.vector.tensor_mul(t1[:], src, cs1_sb[:, sl])
                    nc.vector.tensor_mul(t2[:], src, cs2_sb[:, sl])
                    nc.scalar.copy(t1s[:], t1[64:128, :])
                    nc.scalar.copy(t2s[:], t2[64:128, :])
                    nc.vector.tensor_add(tens[0:64, h, sl], t1[0:64, :], t1s[:])
                    nc.vector.tensor_add(
                        tens[64:128, h, sl], t2[0:64, :], t2s[:]
                    )

            def emit_qkv(b):
                tb = b * S
                for ch in range(NCHK_B):
                    t0 = tb + ch * CHK
                    xc = x_pool.tile([128, NDT, CHK], F32R, tag="xc")
                    nc.sync.dma_start(xc[:], xT_re[:, :, t0 : t0 + CHK])
                    for h in range(HLOC):
                        psq = ps_qk.tile([128, CHK], F32, tag="psqk")
                        psk = ps_qk.tile([128, CHK], F32, tag="psqk")
                        for dt in range(NDT):
                            nc.tensor.matmul(
                                psq[:],
                                lhsT=wq_sb[:, dt, h * HD : (h + 1) * HD],
                                rhs=xc[:, dt, :],
                                start=(dt == 0),
                                stop=(dt == NDT - 1),
                            )
                        for dt in range(NDT):
                            nc.tensor.matmul(
                                psk[:],
                                lhsT=wk_sb[:, dt, h * HD : (h + 1) * HD],
                                rhs=xc[:, dt, :],
                                start=(dt == 0),
                                stop=(dt == NDT - 1),
                            )
                        nc.vector.tensor_copy(
                            q_sb[:, h, ch * CHK : (ch + 1) * CHK], psq[:]
                        )
                        nc.vector.tensor_copy(
                            k_sb[:, h, ch * CHK : (ch + 1) * CHK], psk[:]
                        )
                    for st in range(CHK // KT):
                        psv = ps_v.tile([128, HLOC * HD], F32, tag="psv")
                        for dt in range(NDT):
                            nc.tensor.matmul(
                                psv[:],
                                lhsT=xc[:, dt, st * KT : (st + 1) * KT],
                                rhs=wv_sb[:, dt, :],
                                start=(dt == 0),
                                stop=(dt == NDT - 1),
                            )
                        nc.vector.tensor_copy(
                            v_sb[:, ch * (CHK // KT) + st, :], psv[:]
                        )
                    if ch == NCHK_B // 2 - 1:
                        for tens in (q_sb, k_sb):
                            emit_rope(tens, 0)
                    elif ch == NCHK_B - 1:
                        for tens in (q_sb, k_sb):
                            emit_rope(tens, 1)

            def emit_attention(b, h):
                tb = b * S
                for tcq in range(NQC_B):
                    q0 = tcq * QC
                    nkt = (tcq + 1) * (QC // KT)
                    acc = small_pool.tile([128, QC], F32R, tag="acc")
                    pv = ps_pv.tile([128, QC], F32, tag="pv")
                    for kt in range(nkt):
                        k0 = kt * KT
                        j = kt - (QC // KT) * tcq
                        ps = ps_sc.tile([128, QC], F32, tag="sc")
                        if j >= 0:
                            # preload additive mask band into PSUM, then
                            # accumulate the scores matmul on top of it
                            nc.tensor.matmul(
                                ps[:],
                                lhsT=id_sb[:],
                                rhs=mband_sb[:, j, :],
                                start=True,
                                stop=False,
                            )
                        nc.tensor.matmul(
                            ps[:],
                            lhsT=k_sb[:, h, k0 : k0 + KT],
                            rhs=q_sb[:, h, q0 : q0 + QC],
                            start=(j < 0),
                            stop=True,
                        )
                        if kt == 0:
                            pt = acc
                            nc.scalar.activation(pt[:], ps[:], EXP, scale=SCALE)
                        else:
                            pt = pt_pool.tile([128, QC], F32R, tag="pt")
                            nc.scalar.activation(pt[:], ps[:], EXP, scale=SCALE)
                            nc.vector.tensor_add(acc[:], acc[:], pt[:])
                        nc.tensor.matmul(
                            pv[:],
                            lhsT=v_sb[:, kt, h * HD : (h + 1) * HD],
                            rhs=pt[:],
                            start=(kt == 0),
                            stop=(kt == nkt - 1),
                        )
                    # broadcast row-sums via all-ones matmul: every output
                    # partition gets l[tq]; then 128-lane reciprocal.
                    lb = ps_sc.tile([128, QC], F32, tag="sc")
                    nc.tensor.matmul(
                        lb[:], lhsT=ones128[:], rhs=acc[:], start=True, stop=True
                    )
                    rbs = small_pool.tile([128, QC], F32, tag="rbs")
                    nc.vector.reciprocal_approx_fast(rbs[:], lb[:])
                    aon = ao_pool.tile([128, QC], BF16, tag="aon")
                    nc.vector.tensor_mul(aon[:], pv[:], rbs[:])
                    nc.sync.dma_start(ao_dram[h][b][:, q0 : q0 + QC], aon[:])
                # head-slot h, batch b fully written -> gather it
                nc.gpsimd.collective_compute(
                    "AllGather",
                    mybir.AluOpType.bypass,
                    replica_groups=[list(range(NCORES))],
                    ins=[ao_dram[h][b].opt()],
                    outs=[g_dram[h][b].opt()],
                )

            # ======== stage 2 helpers ========
            s2_open = {}

            def emit_stage2(b, s2_pools):
                s2_pool, g_pool, s2_psum = s2_pools
                wog_sb = s2_open.get("wog_sb")
                if wog_sb is None:
                    wog_sb = s2_pool.tile([128, NDT, 256], BF16, name="wog_sb")
                    nc.sync.dma_start(wog_sb[:], wog_re)
                    s2_open["wog_sb"] = wog_sb
                for tcg in range(NQC_B):
                    gts = []
                    for ad in range(NDT):
                        gt = g_pool.tile(
                            [128, QC],
                            BF16,
                            tag="gt",
                            bufs=18,
                            name=f"gt{b}_{tcg}_{ad}",
                        )
                        gsrc = g_dram[ad // NCORES][b]
                        row0 = (ad % NCORES) * 128
                        nc.sync.dma_start(
                            gt[:],
                            gsrc[row0 : row0 + 128, tcg * QC : (tcg + 1) * QC],
                        )
                        gts.append(gt)
                    for od in range(2):
                        pso = s2_psum.tile(
                            [128, QC], F32, tag="so", name=f"so{b}_{tcg}_{od}"
                        )
                        for ad in range(NDT):
                            nc.tensor.matmul(
                                pso[:],
                                lhsT=wog_sb[:, ad, od * 128 : (od + 1) * 128],
                                rhs=gts[ad][:],
                                start=(ad == 0),
                                stop=(ad == NDT - 1),
                            )
                        ost = g_pool.tile(
                            [128, QC], F32, tag="ost", name=f"ost{b}_{tcg}_{od}"
                        )
                        nc.vector.tensor_copy(ost[:], pso[:])
                        nc.sync.dma_start(
                            out.ap()[
                                od * 128 : (od + 1) * 128,
                                b * S + tcg * QC : b * S + (tcg + 1) * QC,
                            ],
                            ost[:],
                        )

            # ======== emission schedule ========
            emit_qkv(0)
            for h in range(HLOC):
                emit_attention(0, h)
            emit_qkv(1)
            # QKV psum pools no longer needed; free banks for stage 2
            qkv_ps_stack.close()
            s2_pool = stack.enter_context(tc.tile_pool(name="s2", bufs=1))
            g_pool = stack.enter_context(tc.tile_pool(name="gp", bufs=2))
            s2_psum = stack.enter_context(
                tc.tile_pool(name="s2ps", bufs=2, space="PSUM")
            )
            s2_pools = (s2_pool, g_pool, s2_psum)
            emit_stage2(0, s2_pools)  # overlaps batch-1 attention
            for h in range(HLOC):
                emit_attention(1, h)
            emit_stage2(1, s2_pools)

    nc.compile()
    return nc


_CACHED = {}


def _get_compiled():
    if "nc" not in _CACHED:
        _CACHED["nc"] = build_kernel()
    return _CACHED["nc"]


def shard_inputs(x, wq, wk, wv, wo, freqs_cos, freqs_sin, mask):
    x = np.asarray(x, np.float32)
    wq = np.asarray(wq, np.float32)
    wk = np.asarray(wk, np.float32)
    wv = np.asarray(wv, np.float32)
    wo = np.asarray(wo, np.float32)
    fc = np.asarray(freqs_cos, np.float32)
    fs = np.asarray(freqs_sin, np.float32)
    mask = np.asarray(mask, np.float32)

    xT = np.ascontiguousarray(x.reshape(T, D).T)  # [D, T]

    # de-interleave within each head: [0,2,...,126, 1,3,...,127]
    perm = np.concatenate([np.arange(0, HD, 2), np.arange(1, HD, 2)])

    ct = fc.T  # [64, S]
    st = fs.T
    cs1 = np.ascontiguousarray(np.concatenate([ct, -st], axis=0))  # [128, S]
    cs2 = np.ascontiguousarray(np.concatenate([st, ct], axis=0))

    # mask bands: band j = sqrt(HD) * mask[0,0, 0:QC, 128j:128(j+1)].T
    m = mask[0, 0]
    mb = np.concatenate(
        [SQHD * m[0:QC, KT * j : KT * (j + 1)].T for j in range(QC // KT)], axis=1
    ).astype(np.float32)  # [128, 4*QC]
    mb = np.ascontiguousarray(mb)

    identity = np.eye(128, dtype=np.float32)

    # wo: gathered row order is [even heads, odd heads] (slot-major)
    woT = wo.T  # [D(ad), D(od)]
    head_order = list(range(0, H, 2)) + list(range(1, H, 2))
    woT_g = woT.reshape(H, HD, D)[head_order].reshape(D, D)

    in_maps = []
    for c in range(NCORES):
        rows = slice(c * HLOC * HD, (c + 1) * HLOC * HD)
        wq_c = wq[rows].reshape(HLOC, HD, D)[:, perm, :].reshape(HLOC * HD, D)
        wk_c = wk[rows].reshape(HLOC, HD, D)[:, perm, :].reshape(HLOC * HD, D)
        in_maps.append(
            {
                "xT": xT,
                "wqp": np.ascontiguousarray(wq_c.T),
                "wkp": np.ascontiguousarray(wk_c.T),
                "wvp": np.ascontiguousarray(wv[rows].T),
                "wog": np.ascontiguousarray(
                    woT_g[:, c * 256 : (c + 1) * 256]
                ).astype(NP_BF16),
                "cs1": cs1,
                "cs2": cs2,
                "mband": mb,
                "ident": identity,
            }
        )
    return in_maps


def run_sharded(in_maps, trace=False):
    nc = _get_compiled()
    res = bass_utils.run_bass_kernel_spmd(
        nc, in_maps, core_ids=list(range(NCORES)), trace=trace
    )
    return res


def unshard(results):
    # results: list of dicts with "out": [256, T]
    out_T = np.concatenate([r["out"] for r in results], axis=0)  # [D, T]
    return np.ascontiguousarray(out_T.T).reshape(B, S, D)


def kernel(**inputs):
    in_maps = shard_inputs(**inputs)
    res = run_sharded(in_maps, trace=False)
    return unshard(res.results)
